# revision 1
# baseline (speedup 1.0000x reference)
"""Trainium2 Bass kernel for the GCN autoencoder problem.

kernel(**inputs) takes the FULL unsharded inputs (x, edge_index, W, b, gamma,
beta), distributes across 8 NeuronCores internally, and returns the full
[12000, 12000] float32 output of:
  GCNConv (self-loops, symmetric norm) -> BatchNorm1d -> ReLU -> z @ z.T

Self-contained: only needs numpy + the concourse (Bass) runtime.
"""


import numpy as np

import concourse.bass as bass
import concourse.bacc as bacc
import concourse.mybir as mybir
import concourse.tile as tile
from concourse.masks import make_identity

N = 12000
C_IN = 256
F = 128
P = 128
NCORES = 8
NW = 12                      # block-slots per core
NLOC = NW * P                # 1536 nodes per shard (padded)
NPAD = NCORES * NLOC         # 12288
NBLK = NPAD // P             # 96 blocks of 128
TPC = 8                      # blocks per residue class (96/12)
NQ = 6                       # dst chunks per core
DQ = 256
BN_EPS = 1e-5

# residue-class tournament: C_k = classes whose columns block-row-slot k computes
CLS = []
for k in range(NW):
    cs = [k] + [(k + d) % NW for d in range(1, 6)]
    if k < 6:
        cs.append((k + 6) % NW)
    CLS.append(sorted(cs))
SLOT_W = [len(c) * TPC * P for c in CLS]          # output cols per slot
SLOT_OFF = np.concatenate([[0], np.cumsum(SLOT_W)]).astype(int)
OUT_W = int(SLOT_OFF[-1])                          # 79872

AF = mybir.ActivationFunctionType
ALU = mybir.AluOpType


# --------------------------------------------------------------------------
# Host-side preprocessing.
# --------------------------------------------------------------------------

def preprocess(x, edge_index, W, gamma, beta):
    src_all = np.concatenate([np.asarray(edge_index[0]), np.arange(N)]).astype(np.int64)
    dst_all = np.concatenate([np.asarray(edge_index[1]), np.arange(N)]).astype(np.int64)
    deg = np.ones(NPAD, dtype=np.float32)
    deg[:N] = np.bincount(dst_all, minlength=N).astype(np.float32)
    dinv = (1.0 / np.sqrt(deg)).astype(np.float32)

    core_of = dst_all // NLOC
    # per (core, q, cblk) cell counts
    cell_q = (dst_all % NLOC) // DQ          # 0..2
    cell_c = src_all // P                    # 0..93
    counts = np.zeros((NCORES, NQ, NBLK), dtype=np.int64)
    np.add.at(counts, (core_of, cell_q, cell_c), 1)
    m_cell = np.maximum(0, (counts.max(axis=0) + P - 1) // P)  # [NQ, NBLK]
    t0 = np.concatenate([[0], np.cumsum(m_cell.ravel())])
    T = int(t0[-1])
    t0 = t0[:-1].reshape(NQ, NBLK)

    x = np.asarray(x, dtype=np.float32)
    xpad = np.zeros((NPAD, C_IN), dtype=np.float32)
    xpad[:N] = x
    W = np.ascontiguousarray(np.asarray(W, dtype=np.float32))
    gamma2 = np.asarray(gamma, dtype=np.float32).reshape(F, 1)
    beta2 = np.asarray(beta, dtype=np.float32).reshape(F, 1)
    deg_t = np.ascontiguousarray(deg.reshape(NBLK, P).T)  # [P, NBLK]

    in_maps = []
    for c in range(NCORES):
        m = core_of == c
        s_c = src_all[m]
        d_c = dst_all[m] % NLOC
        q_c = d_c // DQ
        cb_c = s_c // P
        order = np.lexsort((cb_c, q_c))
        s_c, d_c, q_c, cb_c = s_c[order], d_c[order], q_c[order], cb_c[order]
        srcr_col = np.zeros((P, T), dtype=np.float32)
        dstr_col = np.full((P, T), -1.0, dtype=np.float32)
        dinvd_col = np.zeros((P, T), dtype=np.float32)
        cnts = counts[c]  # [NQ, NBLK]
        off = 0
        for q in range(NQ):
            for cb in range(NBLK):
                n_e = cnts[q, cb]
                if n_e == 0:
                    continue
                sl = slice(off, off + n_e)
                off += n_e
                j = np.arange(n_e)
                cols = int(t0[q, cb]) + j // P
                parts = j % P
                srcr_col[parts, cols] = (s_c[sl] % P).astype(np.float32)
                dstr_col[parts, cols] = (d_c[sl] % DQ).astype(np.float32)
                dinvd_col[parts, cols] = dinv[(d_c[sl] + c * NLOC)]
        in_maps.append({
            "x_full": xpad,
            "Wt": W,
            "gamma": gamma2,
            "beta": beta2,
            "deg_t": deg_t,
            "srcr_col": srcr_col,
            "dstr_col": dstr_col,
            "dinvd_col": dinvd_col,
        })
    return in_maps, m_cell.tolist()


# --------------------------------------------------------------------------
# Device program (one SPMD program for all 8 cores).
# --------------------------------------------------------------------------

def build(m_cell, bench_phase=None, bench_r=16):
    m_cell = np.asarray(m_cell, dtype=np.int64)
    T = int(m_cell.sum())
    nc = bacc.Bacc("TRN2", target_bir_lowering=False, debug=False,
                   num_devices=NCORES)
    f32 = mybir.dt.float32
    fp16 = mybir.dt.float16
    i32 = mybir.dt.int32

    x_full = nc.dram_tensor("x_full", [NPAD, C_IN], f32, kind="ExternalInput")
    Wt = nc.dram_tensor("Wt", [C_IN, F], f32, kind="ExternalInput")
    gamma = nc.dram_tensor("gamma", [F, 1], f32, kind="ExternalInput")
    beta = nc.dram_tensor("beta", [F, 1], f32, kind="ExternalInput")
    deg_t = nc.dram_tensor("deg_t", [P, NBLK], f32, kind="ExternalInput")
    srcr_col = nc.dram_tensor("srcr_col", [P, T], f32, kind="ExternalInput")
    dstr_col = nc.dram_tensor("dstr_col", [P, T], f32, kind="ExternalInput")
    dinvd_col = nc.dram_tensor("dinvd_col", [P, T], f32, kind="ExternalInput")
    out = nc.dram_tensor("out", [P, OUT_W], f32, kind="ExternalOutput")

    rg = [list(range(NCORES))]

    with tile.TileContext(nc) as tc:
        with tc.tile_pool(name="const", bufs=1) as const, \
             tc.tile_pool(name="big", bufs=1) as big, \
             tc.tile_pool(name="xwp_pool", bufs=1) as xwp_pool, \
             tc.tile_pool(name="dram", bufs=1, space="DRAM") as dram:
            # ---------------- constants ----------------
            ident = const.tile([P, P], f32)
            make_identity(nc, ident[:])
            iota_i = const.tile([P, DQ], i32)
            nc.gpsimd.iota(iota_i[:], pattern=[[1, DQ]], base=0,
                           channel_multiplier=0)
            iota_h = const.tile([P, DQ], fp16)
            nc.vector.tensor_copy(iota_h[:], iota_i[:])
            W_sb = const.tile([P, 2 * P], f32)
            nc.sync.dma_start(W_sb[:, 0:P], Wt[0:P, :])
            nc.sync.dma_start(W_sb[:, P:2 * P], Wt[P:2 * P, :])
            gamma_sb = const.tile([P, 1], f32)
            nc.sync.dma_start(gamma_sb[:], gamma[:, :])
            beta_sb = const.tile([P, 1], f32)
            nc.sync.dma_start(beta_sb[:], beta[:, :])
            deg_sb = const.tile([P, NBLK], f32)
            nc.sync.dma_start(deg_sb[:], deg_t[:, :])
            dinv_sb = const.tile([P, NBLK], f32)
            nc.scalar.activation(dinv_sb[:], deg_sb[:], AF.Sqrt)
            nc.vector.reciprocal(dinv_sb[:], dinv_sb[:])
            srcr_sb = const.tile([P, T], f32)
            nc.sync.dma_start(srcr_sb[:], srcr_col[:, :])
            dstr_sb = const.tile([P, T], f32)
            nc.sync.dma_start(dstr_sb[:], dstr_col[:, :])
            dinvd_sb = const.tile([P, T], f32)
            nc.sync.dma_start(dinvd_sb[:], dinvd_col[:, :])

            # persistent tiles
            hT = big.tile([P, NLOC], f32)          # local h, feature-major
            zT_loc = big.tile([P, NLOC], fp16)
            zT_full = big.tile([P, NPAD], fp16)
            xw_blks = [xwp_pool.tile([P, P], fp16, tag=f"xw{c}",
                                     name=f"xwb{c}") for c in range(NBLK)]

            ag2_in = dram.tile([P, NLOC], f32)
            ag2_out = dram.tile([NCORES * P, NLOC], f32, addr_space="Shared")

            # ------- phase 1: xw' = dinv * (x @ W), all 96 blocks, fp16 ------
            with tc.tile_pool(name="p1", bufs=4) as p1, \
                 tc.tile_pool(name="p1ps", bufs=2, space="PSUM") as p1ps:
              def phase1():
                for r in range(NCORES):
                    xsb = p1.tile([P, NW * C_IN], f32, tag="xsb")
                    xsrc = x_full[r * NLOC:(r + 1) * NLOC, :].rearrange(
                        "(b p) c -> p b c", p=P)
                    nc.sync.dma_start(
                        xsb[:].rearrange("p (b c) -> p b c", b=NW), xsrc)
                    for bb in range(NW):
                        mt = r * NW + bb
                        xb = xsb[:, bb * C_IN:(bb + 1) * C_IN]
                        xw_ps = p1ps.tile([P, P], f32, tag="xwps")
                        for cb in range(2):
                            tps = p1ps.tile([P, P], f32, tag="tps")
                            nc.tensor.transpose(tps[:],
                                                xb[:, cb * P:(cb + 1) * P],
                                                ident[:])
                            xT = p1.tile([P, P], f32, tag="xT")
                            nc.scalar.copy(xT[:], tps[:])
                            nc.tensor.matmul(xw_ps[:], lhsT=xT[:],
                                             rhs=W_sb[:, cb * P:(cb + 1) * P],
                                             start=(cb == 0), stop=(cb == 1))
                        nc.scalar.activation(xw_blks[mt][:], xw_ps[:], AF.Copy,
                                             scale=dinv_sb[:, mt:mt + 1])
              if bench_phase == "xw":
                  with tc.For_i(0, bench_r, 1):
                      phase1()
              else:
                  phase1()

            # ------- phase 3: conv via cell outer-product matmuls ------------
            with tc.tile_pool(name="p3", bufs=8) as p3, \
                 tc.tile_pool(name="p3w", bufs=3) as p3w, \
                 tc.tile_pool(name="p3ps", bufs=4, space="PSUM") as p3ps, \
                 tc.tile_pool(name="p3ph", bufs=1, space="PSUM") as p3ph:
              def phase3():
                t = 0
                for q in range(NQ):
                    hps = p3ph.tile([P, DQ], f32, tag="hps")
                    live = [c for c in range(NBLK) if m_cell[q][c] > 0]
                    for c in live:
                        ohps = p3ps.tile([P, DQ], f32, tag="ohps")
                        mk = int(m_cell[q][c])
                        for k in range(mk):
                            ohs = p3.tile([P, P], fp16, tag="ohs")
                            nc.vector.tensor_scalar(
                                out=ohs[:], in0=iota_h[:, 0:P],
                                scalar1=srcr_sb[:, t:t + 1], scalar2=None,
                                op0=ALU.is_equal)
                            ohd = p3.tile([P, DQ], fp16, tag="ohd")
                            nc.vector.tensor_scalar(
                                out=ohd[:], in0=iota_h[:],
                                scalar1=dstr_sb[:, t:t + 1],
                                scalar2=dinvd_sb[:, t:t + 1],
                                op0=ALU.is_equal, op1=ALU.mult)
                            nc.tensor.matmul(ohps[:], lhsT=ohs[:], rhs=ohd[:],
                                             start=(k == 0), stop=(k == mk - 1))
                            t += 1
                        oh_sb = p3w.tile([P, DQ], fp16, tag="ohsb")
                        nc.scalar.copy(oh_sb[:], ohps[:])
                        nc.tensor.matmul(hps[:], lhsT=xw_blks[c][:],
                                         rhs=oh_sb[:],
                                         start=(c == live[0]),
                                         stop=(c == live[-1]))
                    nc.scalar.copy(hT[:, q * DQ:(q + 1) * DQ], hps[:])
              if bench_phase == "conv":
                  with tc.For_i(0, bench_r, 1):
                      phase3()
              else:
                  phase3()

            # ---------------- phase 4+5: AllGather hT, batchnorm, relu -------
            with tc.tile_pool(name="p5", bufs=1) as p5:
                nc.sync.dma_start(ag2_in[:, :], hT[:])
                nc.gpsimd.collective_compute(
                    "AllGather", ALU.bypass, replica_groups=rg,
                    ins=[ag2_in.opt()], outs=[ag2_out.opt()])
                hT_full = p5.tile([P, NPAD], f32)
                for r in range(NCORES):
                    nc.sync.dma_start(hT_full[:, r * NLOC:(r + 1) * NLOC],
                                      ag2_out[r * P:(r + 1) * P, :])
                # stats over all real nodes (pad columns are exactly zero)
                ssum = p5.tile([P, 1], f32)
                nc.vector.reduce_sum(out=ssum[:], in_=hT_full[:],
                                     axis=mybir.AxisListType.X)
                ssq_p = p5.tile([P, NCORES], f32)
                sq = p5.tile([P, NLOC], f32)
                for r in range(NCORES):
                    nc.scalar.activation(sq[:],
                                         hT_full[:, r * NLOC:(r + 1) * NLOC],
                                         AF.Square,
                                         accum_out=ssq_p[:, r:r + 1])
                ssq = p5.tile([P, 1], f32)
                nc.vector.reduce_sum(out=ssq[:], in_=ssq_p[:],
                                     axis=mybir.AxisListType.X)
                mean = p5.tile([P, 1], f32)
                nc.vector.tensor_scalar_mul(mean[:], ssum[:], 1.0 / N)
                ex2 = p5.tile([P, 1], f32)
                nc.vector.tensor_scalar_mul(ex2[:], ssq[:], 1.0 / N)
                m2 = p5.tile([P, 1], f32)
                nc.vector.tensor_mul(m2[:], mean[:], mean[:])
                var = p5.tile([P, 1], f32)
                nc.vector.tensor_tensor(out=var[:], in0=ex2[:], in1=m2[:],
                                        op=ALU.subtract)
                eps_sb = p5.tile([P, 1], f32)
                nc.gpsimd.memset(eps_sb[:], BN_EPS)
                sd = p5.tile([P, 1], f32)
                nc.scalar.activation(sd[:], var[:], AF.Sqrt,
                                     bias=eps_sb[:, :1])
                rstd = p5.tile([P, 1], f32)
                nc.vector.reciprocal(rstd[:], sd[:])
                scale_f = p5.tile([P, 1], f32)
                nc.vector.tensor_mul(scale_f[:], rstd[:], gamma_sb[:])
                msc = p5.tile([P, 1], f32)
                nc.vector.tensor_mul(msc[:], mean[:], scale_f[:])
                shift_f = p5.tile([P, 1], f32)
                nc.vector.tensor_tensor(out=shift_f[:], in0=beta_sb[:],
                                        in1=msc[:], op=ALU.subtract)
                nc.scalar.activation(zT_loc[:], hT[:], AF.Relu,
                                     bias=shift_f[:, :1], scale=scale_f[:, :1])
                nc.scalar.activation(zT_full[:], hT_full[:], AF.Relu,
                                     bias=shift_f[:, :1], scale=scale_f[:, :1])

            # ---------------- phase 6: decode z @ z.T (upper classes) --------
            zv = zT_full[:].rearrange("p (t c) -> p t c", t=TPC)
            with tc.tile_pool(name="p6", bufs=2) as p6, \
                 tc.tile_pool(name="p6ps", bufs=4, space="PSUM") as p6ps:
              def phase6():
                for k in range(NW):
                    ob = p6.tile([P, max(SLOT_W)], f32, tag="ob")
                    for ci, m in enumerate(CLS[k]):
                        for half in range(2):
                            ops = p6ps.tile([P, 512], f32, tag="ops")
                            rhs = zv[:, half * 4:(half + 1) * 4,
                                     m * P:(m + 1) * P]
                            nc.tensor.matmul(
                                ops[:],
                                lhsT=zT_loc[:, k * P:(k + 1) * P],
                                rhs=rhs, start=True, stop=True)
                            o0 = ci * TPC * P + half * 512
                            if (ci + half) % 2 == 0:
                                nc.vector.tensor_copy(
                                    ob[:, o0:o0 + 512], ops[:])
                            else:
                                nc.scalar.copy(ob[:, o0:o0 + 512], ops[:])
                    nc.sync.dma_start(
                        out[:, int(SLOT_OFF[k]):int(SLOT_OFF[k]) + SLOT_W[k]],
                        ob[:, :SLOT_W[k]])
              if bench_phase == "dec":
                  for _ in range(bench_r):
                      phase6()
              else:
                  phase6()
    nc.compile()
    return nc


# --------------------------------------------------------------------------
# Host-side unsharding: unpack class-layout, mirror the missing triangle.
# --------------------------------------------------------------------------

def assemble_output(results):
    full = np.zeros((NPAD, NPAD), dtype=np.float32)
    filled = np.zeros((NBLK, NBLK), dtype=bool)
    for c in range(NCORES):
        o = results[c]["out"]  # [P, OUT_W]
        for k in range(NW):
            a = c * NW + k  # global row block
            slot = o[:, int(SLOT_OFF[k]):int(SLOT_OFF[k]) + SLOT_W[k]]
            for ci, m in enumerate(CLS[k]):
                for t in range(TPC):
                    b = t * NW + m  # global col block
                    full[a * P:(a + 1) * P, b * P:(b + 1) * P] = \
                        slot[:, ci * TPC * P + t * P:(ci * TPC + t + 1) * P]
                    filled[a, b] = True
    for a in range(NBLK):
        for b in range(NBLK):
            if not filled[a, b]:
                full[a * P:(a + 1) * P, b * P:(b + 1) * P] = \
                    full[b * P:(b + 1) * P, a * P:(a + 1) * P].T
    return np.ascontiguousarray(full[:N, :N])


from concourse import bass_utils

_CACHE = {}


def kernel(x, edge_index, W, b, gamma, beta):
    in_maps, m_cell = preprocess(x, edge_index, W, gamma, beta)
    key = tuple(tuple(r) for r in m_cell)
    if key not in _CACHE:
        _CACHE[key] = build(m_cell)
    nc = _CACHE[key]
    res = bass_utils.run_bass_kernel_spmd(
        nc, in_maps, core_ids=list(range(NCORES)))
    return assemble_output(res.results)



# revision 2
# speedup vs baseline: 3.3046x; 3.3046x over previous
"""Trainium2 Bass kernel for the GCN autoencoder problem.

kernel(**inputs) takes the FULL unsharded inputs (x, edge_index, W, b, gamma,
beta), distributes across 8 NeuronCores internally, and returns the full
[12000, 12000] float32 output of:
  GCNConv (self-loops, symmetric norm) -> BatchNorm1d -> ReLU -> z @ z.T

Strategy (v2, dense-adjacency): the scatter-add is reformulated as a dense
matmul hT = xw'.T @ M where M[s, d] is the edge-multiplicity matrix shipped
as fp8_e4m3 (small integers -> exact).  The symmetric normalization
dinv[s]*dinv[d] is folded into xw' (src side, per-partition scale) and a
post-matmul column scale (dst side).  Each core owns 1536 dst nodes; h is
AllGathered (fp16, pipelined in 3 chunks), BN stats computed on device, and
the z @ z.T decode runs a block-tournament so only ~57% of the symmetric
output is computed and written (fp16); the host mirrors the rest.

Self-contained: only needs numpy + ml_dtypes + the concourse (Bass) runtime.
"""

import numpy as np
import ml_dtypes

import concourse.bass as bass
import concourse.bacc as bacc
import concourse.mybir as mybir
import concourse.tile as tile

N = 12000
C_IN = 256
F = 128
P = 128
NCORES = 8
NW = 12                      # 128-blocks per core (rows)
NLOC = NW * P                # 1536 nodes per shard (padded)
NPAD = NCORES * NLOC         # 12288
NBLK = NPAD // P             # 96 blocks of 128
TPC = 8                      # blocks per residue class (96/12)
NQ = 3                       # dst chunks per core
DQ = 512
SBATCH = 12                  # src-blocks per A DMA batch
BN_EPS = 1e-5
AT_COLS = NQ * NBLK * DQ     # 147456

# residue-class tournament: C_k = classes whose columns block-row-slot k computes
CLS = []
for k in range(NW):
    cs = [k] + [(k + d) % NW for d in range(1, 6)]
    if k < 6:
        cs.append((k + 6) % NW)
    CLS.append(sorted(cs))
SLOT_W = [len(c) * TPC * P for c in CLS]          # output cols per slot
SLOT_OFF = np.concatenate([[0], np.cumsum(SLOT_W)]).astype(int)
OUT_W = int(SLOT_OFF[-1])                          # 79872

AF = mybir.ActivationFunctionType
ALU = mybir.AluOpType


# --------------------------------------------------------------------------
# Host-side preprocessing: indices -> dense fp8 multiplicity matrix + scales.
# --------------------------------------------------------------------------

def preprocess(x, edge_index, W, gamma, beta):
    src = np.asarray(edge_index[0]).astype(np.int64)
    dst = np.asarray(edge_index[1]).astype(np.int64)
    src_all = np.concatenate([src, np.arange(N, dtype=np.int64)])
    dst_all = np.concatenate([dst, np.arange(N, dtype=np.int64)])
    deg = np.bincount(dst_all, minlength=N).astype(np.float32)
    dinv_pad = np.ones(NPAD, dtype=np.float32)
    dinv_pad[:N] = 1.0 / np.sqrt(deg)

    counts = np.zeros((NPAD, NPAD), dtype=np.uint8)
    np.add.at(counts, (src_all, dst_all), 1)
    M8 = counts.astype(ml_dtypes.float8_e4m3)     # exact small ints

    xpad = np.zeros((NPAD, C_IN), dtype=np.float16)
    xpad[:N] = np.asarray(x, dtype=np.float32).astype(np.float16)
    xT = np.ascontiguousarray(xpad.T)             # [256, NPAD] fp16
    W16 = np.asarray(W, dtype=np.float32).astype(np.float16)  # [256, 128]
    gamma2 = np.asarray(gamma, dtype=np.float32).reshape(F, 1)
    beta2 = np.asarray(beta, dtype=np.float32).reshape(F, 1)
    dinv_sb = np.ascontiguousarray(dinv_pad.reshape(NBLK, P).T)  # [128, 96]

    in_maps = []
    for c in range(NCORES):
        Ml = M8[:, c * NLOC:(c + 1) * NLOC]
        A_packed = np.ascontiguousarray(
            Ml.reshape(NBLK, P, NQ, DQ).transpose(1, 2, 0, 3)
              .reshape(P, AT_COLS))
        dinvd = np.ascontiguousarray(np.broadcast_to(
            dinv_pad[c * NLOC:(c + 1) * NLOC], (P, NLOC)).astype(np.float32))
        in_maps.append({
            "xT": xT,
            "Wt": W16,
            "gamma": gamma2,
            "beta": beta2,
            "dinv_sb": dinv_sb,
            "A_packed": A_packed,
            "dinvd": dinvd,
        })
    return in_maps, None


# --------------------------------------------------------------------------
# Device program (one SPMD program for all 8 cores).
# --------------------------------------------------------------------------

def build(meta=None, bench_phase=None, bench_r=8):
    nc = bacc.Bacc("TRN2", target_bir_lowering=False, debug=False,
                   num_devices=NCORES)
    f32 = mybir.dt.float32
    fp16 = mybir.dt.float16
    fp8 = mybir.dt.float8e4

    xT_d = nc.dram_tensor("xT", [C_IN, NPAD], fp16, kind="ExternalInput")
    Wt_d = nc.dram_tensor("Wt", [C_IN, F], fp16, kind="ExternalInput")
    gamma_d = nc.dram_tensor("gamma", [F, 1], f32, kind="ExternalInput")
    beta_d = nc.dram_tensor("beta", [F, 1], f32, kind="ExternalInput")
    dinv_d = nc.dram_tensor("dinv_sb", [P, NBLK], f32, kind="ExternalInput")
    A_d = nc.dram_tensor("A_packed", [P, AT_COLS], fp8, kind="ExternalInput")
    dinvd_d = nc.dram_tensor("dinvd", [P, NLOC], f32, kind="ExternalInput")
    out_d = nc.dram_tensor("out", [P, OUT_W], fp16, kind="ExternalOutput")

    rg = [list(range(NCORES))]

    with tile.TileContext(nc) as tc:
        with tc.tile_pool(name="const", bufs=1) as const, \
             tc.tile_pool(name="big", bufs=1) as big, \
             tc.tile_pool(name="dram", bufs=1, space="DRAM") as dram:
            # ---------------- constants ----------------
            W_sb = const.tile([P, 2 * F], fp16)
            nc.sync.dma_start(W_sb[:, 0:F], Wt_d[0:P, :])
            nc.sync.dma_start(W_sb[:, F:2 * F], Wt_d[P:2 * P, :])
            gamma_sb = const.tile([P, 1], f32)
            nc.sync.dma_start(gamma_sb[:], gamma_d[:, :])
            beta_sb = const.tile([P, 1], f32)
            nc.sync.dma_start(beta_sb[:], beta_d[:, :])
            dinv_sb = const.tile([P, NBLK], f32)
            nc.sync.dma_start(dinv_sb[:], dinv_d[:, :])
            dinvd_sb = const.tile([P, NLOC], f32)
            nc.sync.dma_start(dinvd_sb[:], dinvd_d[:, :])

            # persistent tiles
            xw_all = big.tile([P, NPAD], fp16)      # xw' blocks, [s-part, f]
            hT_loc = big.tile([P, NLOC], fp16)      # local h, feature-major
            hT_full = big.tile([P, NPAD], fp16)
            zT_loc = big.tile([P, NLOC], fp16)
            zT_full = big.tile([P, NPAD], fp16)
            sq_scr = big.tile([P, TPC * DQ], fp16)  # Square scratch
            ssum_p = big.tile([P, NQ], f32)
            ssq_p = big.tile([P, NQ], f32)

            ag_in = [dram.tile([P, DQ], fp16, name=f"agi{q}")
                     for q in range(NQ)]
            ag_out = [dram.tile([NCORES * P, DQ], fp16, addr_space="Shared",
                                name=f"ago{q}") for q in range(NQ)]

            # ------- phase X: xw' = dinv[s] * (x @ W), all 96 blocks --------
            with tc.tile_pool(name="px", bufs=2) as px, \
                 tc.tile_pool(name="pxps", bufs=2, space="PSUM") as pxps:
              def phasex():
                for g in range(8):
                    xs0 = px.tile([P, NW * P], fp16, tag="xs0")
                    nc.sync.dma_start(
                        xs0[:], xT_d[0:P, g * NLOC:(g + 1) * NLOC])
                    xs1 = px.tile([P, NW * P], fp16, tag="xs1")
                    nc.sync.dma_start(
                        xs1[:], xT_d[P:2 * P, g * NLOC:(g + 1) * NLOC])
                    for b in range(NW):
                        s = g * NW + b
                        xwps = pxps.tile([P, F], f32, tag="xwps")
                        nc.tensor.matmul(xwps[:],
                                         lhsT=xs0[:, b * P:(b + 1) * P],
                                         rhs=W_sb[:, 0:F],
                                         start=True, stop=False)
                        nc.tensor.matmul(xwps[:],
                                         lhsT=xs1[:, b * P:(b + 1) * P],
                                         rhs=W_sb[:, F:2 * F],
                                         start=False, stop=True)
                        nc.scalar.activation(xw_all[:, s * P:(s + 1) * P],
                                             xwps[:], AF.Copy,
                                             scale=dinv_sb[:, s:s + 1])
              if bench_phase == "xw":
                  with tc.For_i(0, bench_r, 1):
                      phasex()
              else:
                  phasex()

            # ------- phase C: hT = xw'.T @ M, dst-chunked, AG pipelined -----
            with tc.tile_pool(name="pa", bufs=4) as pa, \
                 tc.tile_pool(name="pcps", bufs=2, space="PSUM") as pcps:
              def conv_chunk(q):
                hps = pcps.tile([P, DQ], f32, tag="hps")
                for batch in range(NBLK // SBATCH):
                    asb = pa.tile([P, SBATCH * DQ], fp8, tag="asb")
                    off = (q * NBLK + batch * SBATCH) * DQ
                    nc.sync.dma_start(asb[:],
                                      A_d[:, off:off + SBATCH * DQ])
                    for j in range(SBATCH):
                        s = batch * SBATCH + j
                        nc.tensor.matmul(hps[:],
                                         lhsT=xw_all[:, s * P:(s + 1) * P],
                                         rhs=asb[:, j * DQ:(j + 1) * DQ],
                                         start=(s == 0), stop=(s == NBLK - 1))
                nc.vector.tensor_tensor(
                    out=hT_loc[:, q * DQ:(q + 1) * DQ], in0=hps[:],
                    in1=dinvd_sb[:, q * DQ:(q + 1) * DQ], op=ALU.mult)
              if bench_phase == "conv":
                  with tc.For_i(0, bench_r, 1):
                      for q in range(NQ):
                          conv_chunk(q)
              else:
                hT_v = hT_full[:].rearrange("p (r q d) -> p r q d",
                                            r=NCORES, q=NQ)
                sq_v = sq_scr[:].rearrange("p (r d) -> p r d", r=NCORES)
                for q in range(NQ):
                    conv_chunk(q)
                    nc.sync.dma_start(ag_in[q][:, :],
                                      hT_loc[:, q * DQ:(q + 1) * DQ])
                    nc.gpsimd.collective_compute(
                        "AllGather", ALU.bypass, replica_groups=rg,
                        ins=[ag_in[q].opt()], outs=[ag_out[q].opt()])
                    for r in range(NCORES):
                        nc.sync.dma_start(
                            hT_full[:, r * NLOC + q * DQ:
                                    r * NLOC + (q + 1) * DQ],
                            ag_out[q][r * P:(r + 1) * P, :])
                    nc.vector.reduce_sum(out=ssum_p[:, q:q + 1],
                                         in_=hT_v[:, :, q, :],
                                         axis=mybir.AxisListType.XY)
                    nc.scalar.activation(sq_v[:], hT_v[:, :, q, :],
                                         AF.Square,
                                         accum_out=ssq_p[:, q:q + 1])

            # ---------------- phase B: batchnorm + relu ----------------------
            with tc.tile_pool(name="pb", bufs=1) as pb:
              def phaseb():
                ssum = pb.tile([P, 1], f32, tag="ssum")
                nc.vector.reduce_sum(out=ssum[:], in_=ssum_p[:],
                                     axis=mybir.AxisListType.X)
                ssq = pb.tile([P, 1], f32, tag="ssq")
                nc.vector.reduce_sum(out=ssq[:], in_=ssq_p[:],
                                     axis=mybir.AxisListType.X)
                mean = pb.tile([P, 1], f32, tag="mean")
                nc.vector.tensor_scalar_mul(mean[:], ssum[:], 1.0 / N)
                ex2 = pb.tile([P, 1], f32, tag="ex2")
                nc.vector.tensor_scalar_mul(ex2[:], ssq[:], 1.0 / N)
                m2 = pb.tile([P, 1], f32, tag="m2")
                nc.vector.tensor_mul(m2[:], mean[:], mean[:])
                var = pb.tile([P, 1], f32, tag="var")
                nc.vector.tensor_tensor(out=var[:], in0=ex2[:], in1=m2[:],
                                        op=ALU.subtract)
                eps_sb = pb.tile([P, 1], f32, tag="eps")
                nc.gpsimd.memset(eps_sb[:], BN_EPS)
                sd = pb.tile([P, 1], f32, tag="sd")
                nc.scalar.activation(sd[:], var[:], AF.Sqrt,
                                     bias=eps_sb[:, :1])
                rstd = pb.tile([P, 1], f32, tag="rstd")
                nc.vector.reciprocal(rstd[:], sd[:])
                scale_f = pb.tile([P, 1], f32, tag="scalef")
                nc.vector.tensor_mul(scale_f[:], rstd[:], gamma_sb[:])
                msc = pb.tile([P, 1], f32, tag="msc")
                nc.vector.tensor_mul(msc[:], mean[:], scale_f[:])
                shift_f = pb.tile([P, 1], f32, tag="shiftf")
                nc.vector.tensor_tensor(out=shift_f[:], in0=beta_sb[:],
                                        in1=msc[:], op=ALU.subtract)
                nc.scalar.activation(zT_loc[:], hT_loc[:], AF.Relu,
                                     bias=shift_f[:, :1], scale=scale_f[:, :1])
                nc.scalar.activation(zT_full[:], hT_full[:], AF.Relu,
                                     bias=shift_f[:, :1], scale=scale_f[:, :1])
              phaseb()

            # ---------------- phase D: decode z @ z.T (upper classes) --------
            zv = zT_full[:].rearrange("p (t c) -> p t c", t=TPC)
            with tc.tile_pool(name="pd", bufs=2) as pd, \
                 tc.tile_pool(name="pdps", bufs=4, space="PSUM") as pdps:
              def phased():
                for k in range(NW):
                    ob = pd.tile([P, max(SLOT_W)], fp16, tag="ob")
                    for ci, m in enumerate(CLS[k]):
                        for half in range(2):
                            ops = pdps.tile([P, 512], f32, tag="ops")
                            rhs = zv[:, half * 4:(half + 1) * 4,
                                     m * P:(m + 1) * P]
                            nc.tensor.matmul(
                                ops[:],
                                lhsT=zT_loc[:, k * P:(k + 1) * P],
                                rhs=rhs, start=True, stop=True)
                            o0 = ci * TPC * P + half * 512
                            if (ci + half) % 2 == 0:
                                nc.vector.tensor_copy(
                                    ob[:, o0:o0 + 512], ops[:])
                            else:
                                nc.scalar.copy(ob[:, o0:o0 + 512], ops[:])
                    nc.sync.dma_start(
                        out_d[:, int(SLOT_OFF[k]):int(SLOT_OFF[k]) + SLOT_W[k]],
                        ob[:, :SLOT_W[k]])
              if bench_phase == "dec":
                  for _ in range(bench_r):
                      phased()
              else:
                  phased()
    nc.compile()
    return nc


# --------------------------------------------------------------------------
# Host-side unsharding: unpack class-layout, mirror the missing triangle.
# --------------------------------------------------------------------------

def assemble_output(results):
    full = np.zeros((NPAD, NPAD), dtype=np.float32)
    filled = np.zeros((NBLK, NBLK), dtype=bool)
    for c in range(NCORES):
        o = results[c]["out"].astype(np.float32)  # [P, OUT_W] fp16 -> f32
        for k in range(NW):
            a = c * NW + k  # global row block
            slot = o[:, int(SLOT_OFF[k]):int(SLOT_OFF[k]) + SLOT_W[k]]
            for ci, m in enumerate(CLS[k]):
                for t in range(TPC):
                    b = t * NW + m  # global col block
                    full[a * P:(a + 1) * P, b * P:(b + 1) * P] = \
                        slot[:, ci * TPC * P + t * P:(ci * TPC + t + 1) * P]
                    filled[a, b] = True
    for a in range(NBLK):
        for b in range(NBLK):
            if not filled[a, b]:
                full[a * P:(a + 1) * P, b * P:(b + 1) * P] = \
                    full[b * P:(b + 1) * P, a * P:(a + 1) * P].T
    return np.ascontiguousarray(full[:N, :N])


from concourse import bass_utils

_CACHE = {}


def kernel(x, edge_index, W, b, gamma, beta):
    in_maps, meta = preprocess(x, edge_index, W, gamma, beta)
    if "nc" not in _CACHE:
        _CACHE["nc"] = build(meta)
    nc = _CACHE["nc"]
    res = bass_utils.run_bass_kernel_spmd(
        nc, in_maps, core_ids=list(range(NCORES)))
    return assemble_output(res.results)


# revision 16
# speedup vs baseline: 7.7400x; 2.3422x over previous
"""Trainium2 Bass kernel for the GCN autoencoder problem.

kernel(**inputs) takes the FULL unsharded inputs (x, edge_index, W, b, gamma,
beta), distributes across 8 NeuronCores internally, and returns the full
[12000, 12000] float32 output of:
  GCNConv (self-loops, symmetric norm) -> BatchNorm1d -> ReLU -> z @ z.T

Strategy (v2, dense-adjacency): the scatter-add is reformulated as a dense
matmul hT = xw'.T @ M where M[s, d] is the edge-multiplicity matrix shipped
as fp8_e4m3 (small integers -> exact).  The symmetric normalization
dinv[s]*dinv[d] is folded into xw' (src side, per-partition scale) and a
post-matmul column scale (dst side).  Each core owns 1536 dst nodes; h is
AllGathered (fp16, pipelined in 3 chunks), BN stats computed on device, and
the z @ z.T decode runs a block-tournament so only ~57% of the symmetric
output is computed and written (fp16); the host mirrors the rest.

Self-contained: only needs numpy + ml_dtypes + the concourse (Bass) runtime.
"""

import numpy as np
import ml_dtypes

import concourse.bass as bass
import concourse.bacc as bacc
import concourse.mybir as mybir
import concourse.tile as tile

N = 12000
C_IN = 256
F = 128
P = 128
NCORES = 8
NW = 12                      # 128-blocks per core (rows)
NLOC = NW * P                # 1536 nodes per shard (padded)
NPAD = NCORES * NLOC         # 12288
NBLK = NPAD // P             # 96 blocks of 128
TPC = 8                      # blocks per residue class (96/12)
NQ = 3                       # dst chunks per core
DQ = 512
SBATCH = 12                  # src-blocks per A DMA batch
BN_EPS = 1e-5
AT_COLS = NQ * NBLK * DQ     # 147456

# residue-class tournament: C_k = classes whose columns block-row-slot k computes
CLS = []
for k in range(NW):
    cs = [k] + [(k + d) % NW for d in range(1, 6)]
    if k < 6:
        cs.append((k + 6) % NW)
    CLS.append(sorted(cs))
SLOT_W = [len(c) * TPC * P for c in CLS]          # output cols per slot
SLOT_OFF = np.concatenate([[0], np.cumsum(SLOT_W)]).astype(int)
OUT_W = int(SLOT_OFF[-1])                          # 79872

AF = mybir.ActivationFunctionType
ALU = mybir.AluOpType


# --------------------------------------------------------------------------
# Host-side preprocessing: indices -> dense fp8 multiplicity matrix + scales.
# --------------------------------------------------------------------------

def preprocess(x, edge_index, W, gamma, beta):
    src = np.asarray(edge_index[0]).astype(np.int64)
    dst = np.asarray(edge_index[1]).astype(np.int64)
    src_all = np.concatenate([src, np.arange(N, dtype=np.int64)])
    dst_all = np.concatenate([dst, np.arange(N, dtype=np.int64)])
    deg = np.bincount(dst_all, minlength=N).astype(np.float32)
    dinv_pad = np.ones(NPAD, dtype=np.float32)
    dinv_pad[:N] = 1.0 / np.sqrt(deg)

    counts = np.zeros((NPAD, NPAD), dtype=np.uint8)
    np.add.at(counts, (src_all, dst_all), 1)
    M8 = counts.astype(ml_dtypes.float8_e4m3)     # exact small ints

    # fold the src-side dinv into x rows (commutes with @W): xw' = (dinv*x)@W
    xpad = np.zeros((NPAD, C_IN), dtype=np.float16)
    xpad[:N] = (np.asarray(x, dtype=np.float32)
                * dinv_pad[:N, None]).astype(np.float16)
    xT = np.ascontiguousarray(xpad.T)             # [256, NPAD] fp16
    W16 = np.asarray(W, dtype=np.float32).astype(np.float16)  # [256, 128]
    gamma2 = np.asarray(gamma, dtype=np.float32).reshape(F, 1)
    beta2 = np.asarray(beta, dtype=np.float32).reshape(F, 1)
    dinv_sb = np.ascontiguousarray(dinv_pad.reshape(NBLK, P).T)  # [128, 96]

    in_maps = []
    for c in range(NCORES):
        Ml = M8[:, c * NLOC:(c + 1) * NLOC]
        A_packed = np.ascontiguousarray(
            Ml.reshape(NBLK, P, NQ, DQ).transpose(1, 2, 0, 3)
              .reshape(P, AT_COLS))
        dinvd = np.ascontiguousarray(np.broadcast_to(
            dinv_pad[c * NLOC:(c + 1) * NLOC], (P, NLOC)).astype(np.float32))
        in_maps.append({
            "xT": xT,
            "Wt": W16,
            "gamma": gamma2,
            "beta": beta2,
            "dinv_sb": dinv_sb,
            "A_packed": A_packed,
            "dinvd": dinvd,
        })
    return in_maps, None


# --------------------------------------------------------------------------
# Device program (one SPMD program for all 8 cores).
# --------------------------------------------------------------------------

def build(meta=None, bench_phase=None, bench_r=8, repeat=1):
    nc = bacc.Bacc("TRN2", target_bir_lowering=False, debug=False,
                   num_devices=NCORES)
    f32 = mybir.dt.float32
    fp16 = mybir.dt.float16
    fp8 = mybir.dt.float8e4

    xT_d = nc.dram_tensor("xT", [C_IN, NPAD], fp16, kind="ExternalInput")
    Wt_d = nc.dram_tensor("Wt", [C_IN, F], fp16, kind="ExternalInput")
    gamma_d = nc.dram_tensor("gamma", [F, 1], f32, kind="ExternalInput")
    beta_d = nc.dram_tensor("beta", [F, 1], f32, kind="ExternalInput")
    dinv_d = nc.dram_tensor("dinv_sb", [P, NBLK], f32, kind="ExternalInput")
    A_d = nc.dram_tensor("A_packed", [P, AT_COLS], fp8, kind="ExternalInput")
    dinvd_d = nc.dram_tensor("dinvd", [P, NLOC], f32, kind="ExternalInput")
    out_d = nc.dram_tensor("out", [P, OUT_W], fp16, kind="ExternalOutput")

    rg = [list(range(NCORES))]

    with tile.TileContext(nc) as tc:
      for rep in range(repeat):
        with tc.tile_pool(name="const", bufs=1) as const, \
             tc.tile_pool(name="big", bufs=1) as big, \
             tc.tile_pool(name="dram", bufs=1, space="DRAM") as dram:
            # ---------------- constants ----------------
            W_sb = const.tile([P, 2 * F], fp16)
            nc.sync.dma_start(W_sb[:, 0:F], Wt_d[0:P, :])
            nc.sync.dma_start(W_sb[:, F:2 * F], Wt_d[P:2 * P, :])
            gamma_sb = const.tile([P, 1], f32)
            nc.sync.dma_start(gamma_sb[:], gamma_d[:, :])
            beta_sb = const.tile([P, 1], f32)
            nc.sync.dma_start(beta_sb[:], beta_d[:, :])
            dinv_sb = const.tile([P, NBLK], f32)
            nc.sync.dma_start(dinv_sb[:], dinv_d[:, :])
            dinvd_sb = const.tile([P, NLOC], f32)
            nc.sync.dma_start(dinvd_sb[:], dinvd_d[:, :])

            # persistent tiles
            xw_all = big.tile([P, NPAD], fp16)      # xw' blocks, [s-part, f]
            hT_loc = big.tile([P, NLOC], fp16)      # local h, feature-major
            hT_full = big.tile([P, NPAD], fp16)
            zT_loc = big.tile([P, NLOC], fp16)
            zT_full = big.tile([P, NPAD], fp16)
            sq_scr = big.tile([P, TPC * DQ], fp16)  # Square scratch
            ssum_p = big.tile([P, NQ], f32)
            ssq_p = big.tile([P, NQ], f32)

            ag_in = [dram.tile([P, DQ], fp16, name=f"agi{q}_{rep}")
                     for q in range(NQ)]
            ag_out = [dram.tile([NCORES * P, DQ], fp16, addr_space="Shared",
                                name=f"ago{q}_{rep}") for q in range(NQ)]

            # ------- phase X: xw' = dinv[s] * (x @ W), all 96 blocks --------
            with tc.tile_pool(name="px", bufs=2) as px, \
                 tc.tile_pool(name="pxps", bufs=2, space="PSUM") as pxps:
              def phasex():
                for g in range(8):
                    xs0 = px.tile([P, NW * P], fp16, tag="xs0")
                    nc.sync.dma_start(
                        xs0[:], xT_d[0:P, g * NLOC:(g + 1) * NLOC])
                    xs1 = px.tile([P, NW * P], fp16, tag="xs1")
                    nc.sync.dma_start(
                        xs1[:], xT_d[P:2 * P, g * NLOC:(g + 1) * NLOC])
                    for bq in range(NW // 4):       # 4 blocks per PSUM tile
                        xwps = pxps.tile([P, 4 * F], f32, tag="xwps")
                        for j in range(4):
                            b = bq * 4 + j
                            nc.tensor.matmul(xwps[:, j * F:(j + 1) * F],
                                             lhsT=xs0[:, b * P:(b + 1) * P],
                                             rhs=W_sb[:, 0:F],
                                             start=True, stop=False)
                            nc.tensor.matmul(xwps[:, j * F:(j + 1) * F],
                                             lhsT=xs1[:, b * P:(b + 1) * P],
                                             rhs=W_sb[:, F:2 * F],
                                             start=False, stop=True)
                        s0 = (g * NW + bq * 4) * P
                        if bq % 2 == 0:
                            nc.scalar.copy(xw_all[:, s0:s0 + 4 * P], xwps[:])
                        else:
                            nc.vector.tensor_copy(xw_all[:, s0:s0 + 4 * P],
                                                  xwps[:])
              if bench_phase == "xw":
                  with tc.For_i(0, bench_r, 1):
                      phasex()
              else:
                  phasex()
            run_rest = bench_phase not in ("xw", "conv")

            # ------- phase C: hT = xw'.T @ M, dst-chunked, AG pipelined -----
            with tc.tile_pool(name="pa", bufs=4) as pa, \
                 tc.tile_pool(name="pcps", bufs=2, space="PSUM") as pcps:
              def conv_chunk(q):
                hps = pcps.tile([P, DQ], f32, tag="hps")
                for batch in range(NBLK // SBATCH):
                    asb = pa.tile([P, SBATCH * DQ], fp8, tag="asb")
                    off = (q * NBLK + batch * SBATCH) * DQ
                    nc.sync.dma_start(asb[:],
                                      A_d[:, off:off + SBATCH * DQ])
                    for j in range(SBATCH):
                        s = batch * SBATCH + j
                        nc.tensor.matmul(hps[:],
                                         lhsT=xw_all[:, s * P:(s + 1) * P],
                                         rhs=asb[:, j * DQ:(j + 1) * DQ],
                                         start=(s == 0), stop=(s == NBLK - 1))
                nc.vector.tensor_tensor(
                    out=hT_loc[:, q * DQ:(q + 1) * DQ], in0=hps[:],
                    in1=dinvd_sb[:, q * DQ:(q + 1) * DQ], op=ALU.mult)
              hT_v = hT_full[:].rearrange("p (r q d) -> p r q d",
                                          r=NCORES, q=NQ)
              sq_v = sq_scr[:].rearrange("p (r d) -> p r d", r=NCORES)

              def ag_chunk(q, agi=None, ago=None):
                agi = ag_in[q] if agi is None else agi
                ago = ag_out[q] if ago is None else ago
                nc.sync.dma_start(agi[:, :],
                                  hT_loc[:, q * DQ:(q + 1) * DQ])
                nc.gpsimd.collective_compute(
                    "AllGather", ALU.bypass, replica_groups=rg,
                    ins=[agi.opt()], outs=[ago.opt()])
                for r in range(NCORES):
                    nc.sync.dma_start(
                        hT_full[:, r * NLOC + q * DQ:
                                r * NLOC + (q + 1) * DQ],
                        ago[r * P:(r + 1) * P, :])
                nc.vector.reduce_sum(out=ssum_p[:, q:q + 1],
                                     in_=hT_v[:, :, q, :],
                                     axis=mybir.AxisListType.XY)
                nc.scalar.activation(sq_v[:], hT_v[:, :, q, :],
                                     AF.Square,
                                     accum_out=ssq_p[:, q:q + 1])
              if bench_phase == "conv":
                  with tc.For_i(0, bench_r, 1):
                      for q in range(NQ):
                          conv_chunk(q)
              elif bench_phase == "xw":
                  pass
              else:
                  for q in range(NQ):
                      conv_chunk(q)
                      ag_chunk(q)
                  if bench_phase == "ag":
                      for it in range(bench_r - 1):
                          for q in range(NQ):
                              agi = dram.tile([P, DQ], fp16,
                                              name=f"bagi{q}_{it}_{rep}")
                              ago = dram.tile([NCORES * P, DQ], fp16,
                                              addr_space="Shared",
                                              name=f"bago{q}_{it}_{rep}")
                              ag_chunk(q, agi, ago)

            # ---------------- phase B: batchnorm + relu ----------------------
            with tc.tile_pool(name="pb", bufs=1) as pb:
              def phaseb():
                ssum = pb.tile([P, 1], f32, tag="ssum")
                nc.vector.reduce_sum(out=ssum[:], in_=ssum_p[:],
                                     axis=mybir.AxisListType.X)
                ssq = pb.tile([P, 1], f32, tag="ssq")
                nc.vector.reduce_sum(out=ssq[:], in_=ssq_p[:],
                                     axis=mybir.AxisListType.X)
                mean = pb.tile([P, 1], f32, tag="mean")
                nc.vector.tensor_scalar_mul(mean[:], ssum[:], 1.0 / N)
                ex2 = pb.tile([P, 1], f32, tag="ex2")
                nc.vector.tensor_scalar_mul(ex2[:], ssq[:], 1.0 / N)
                m2 = pb.tile([P, 1], f32, tag="m2")
                nc.vector.tensor_mul(m2[:], mean[:], mean[:])
                var = pb.tile([P, 1], f32, tag="var")
                nc.vector.tensor_tensor(out=var[:], in0=ex2[:], in1=m2[:],
                                        op=ALU.subtract)
                eps_sb = pb.tile([P, 1], f32, tag="eps")
                nc.gpsimd.memset(eps_sb[:], BN_EPS)
                sd = pb.tile([P, 1], f32, tag="sd")
                nc.scalar.activation(sd[:], var[:], AF.Sqrt,
                                     bias=eps_sb[:, :1])
                rstd = pb.tile([P, 1], f32, tag="rstd")
                nc.vector.reciprocal(rstd[:], sd[:])
                scale_f = pb.tile([P, 1], f32, tag="scalef")
                nc.vector.tensor_mul(scale_f[:], rstd[:], gamma_sb[:])
                msc = pb.tile([P, 1], f32, tag="msc")
                nc.vector.tensor_mul(msc[:], mean[:], scale_f[:])
                shift_f = pb.tile([P, 1], f32, tag="shiftf")
                nc.vector.tensor_tensor(out=shift_f[:], in0=beta_sb[:],
                                        in1=msc[:], op=ALU.subtract)
                nc.scalar.activation(zT_loc[:], hT_loc[:], AF.Relu,
                                     bias=shift_f[:, :1], scale=scale_f[:, :1])
                nc.scalar.activation(zT_full[:], hT_full[:], AF.Relu,
                                     bias=shift_f[:, :1], scale=scale_f[:, :1])
              if run_rest:
                  phaseb()

            # ---------------- phase D: decode z @ z.T (upper classes) --------
            zv = zT_full[:].rearrange("p (t c) -> p t c", t=TPC)
            with tc.tile_pool(name="pd", bufs=2) as pd, \
                 tc.tile_pool(name="pdps", bufs=4, space="PSUM") as pdps:
              def phased():
                for k in range(NW):
                    ob = pd.tile([P, max(SLOT_W)], fp16, tag="ob")
                    for ci, m in enumerate(CLS[k]):
                        for half in range(2):
                            ops = pdps.tile([P, 512], f32, tag="ops")
                            rhs = zv[:, half * 4:(half + 1) * 4,
                                     m * P:(m + 1) * P]
                            nc.tensor.matmul(
                                ops[:],
                                lhsT=zT_loc[:, k * P:(k + 1) * P],
                                rhs=rhs, start=True, stop=True)
                            o0 = ci * TPC * P + half * 512
                            if (ci + half) % 2 == 0:
                                nc.vector.tensor_copy(
                                    ob[:, o0:o0 + 512], ops[:])
                            else:
                                nc.scalar.copy(ob[:, o0:o0 + 512], ops[:])
                    nc.sync.dma_start(
                        out_d[:, int(SLOT_OFF[k]):int(SLOT_OFF[k]) + SLOT_W[k]],
                        ob[:, :SLOT_W[k]])
              if bench_phase == "dec":
                  for _ in range(bench_r):
                      phased()
              elif run_rest:
                  phased()
    nc.compile()
    return nc


# --------------------------------------------------------------------------
# Host-side unsharding: unpack class-layout, mirror the missing triangle.
# --------------------------------------------------------------------------

def assemble_output(results):
    full = np.zeros((NPAD, NPAD), dtype=np.float32)
    filled = np.zeros((NBLK, NBLK), dtype=bool)
    for c in range(NCORES):
        o = results[c]["out"].astype(np.float32)  # [P, OUT_W] fp16 -> f32
        for k in range(NW):
            a = c * NW + k  # global row block
            slot = o[:, int(SLOT_OFF[k]):int(SLOT_OFF[k]) + SLOT_W[k]]
            for ci, m in enumerate(CLS[k]):
                for t in range(TPC):
                    b = t * NW + m  # global col block
                    full[a * P:(a + 1) * P, b * P:(b + 1) * P] = \
                        slot[:, ci * TPC * P + t * P:(ci * TPC + t + 1) * P]
                    filled[a, b] = True
    for a in range(NBLK):
        for b in range(NBLK):
            if not filled[a, b]:
                full[a * P:(a + 1) * P, b * P:(b + 1) * P] = \
                    full[b * P:(b + 1) * P, a * P:(a + 1) * P].T
    return np.ascontiguousarray(full[:N, :N])


from concourse import bass_utils

_CACHE = {}


def kernel(x, edge_index, W, b, gamma, beta):
    in_maps, meta = preprocess(x, edge_index, W, gamma, beta)
    if "nc" not in _CACHE:
        _CACHE["nc"] = build(meta)
    nc = _CACHE["nc"]
    res = bass_utils.run_bass_kernel_spmd(
        nc, in_maps, core_ids=list(range(NCORES)))
    return assemble_output(res.results)
